# revision 20
# baseline (speedup 1.0000x reference)
"""Trainium2 Bass kernel for a dense transformer encoder layer.

Problem: B=2, S=2048, D=1024, H=16 heads (W=64), F=4096, fp32 in/out.

Sharding: 8 cores = 2 batches x 4 sequence chunks of 512 tokens. Each core
computes K/V for its batch's full sequence and Q/attention/FFN for its own
512-token chunk. No collectives.

Precision plan (rel-err budget 2e-2, measured ~2e-3):
- K/V/Q projections, attn-V and the softmax normalizer Z run in fp8 e4m3
  with DoubleRow matmuls (0.5 cyc/row, 256-deep contraction). Attention's
  contribution to the residual stream is ~1% of its magnitude, so fp8
  noise there is suppressed ~100x.
- scores and out-proj run in fp8 without DoubleRow (score contraction is
  only 64 deep and out-proj wants 128 output rows; DR's M<=64 limit gives
  no win there).
- FFN runs in bf16 (its output is ~0.4x of the stream - fp8 would cost ~2%).
- LayerNorm path stays fp32 (f32r matmuls for stats + rank-1 apply).

Layouts: activations transposed ([feature, token]); kT/qT live on SBUF
partitions 0-63 (DR outputs must start at partition 0 with M<=64), one
64-feature chunk per head, so scores for head h need only K/Q chunk h -
the exp stream starts ~5us into the kernel. V ([token, feature]) and hT
halves on partitions 64-127 are placed via SBUF->SBUF DMA. Z rides a
second DR matmul with a replicated-ones stationary (matmul cost depends
only on moving size, so 64-row replication is free) and normalization is
one DVE reciprocal + one multiply per head.

Schedule: per head h: K-chunk(h), Q-chunk(h), scores(h) -> exp(h) on Act;
V tiles are packed into the first 5 head-iterations and attn-V/Z/normalize
is deferred 5 heads (probs pool holds 6 heads x 8KB) so PE and Act both
stay busy through the ~150us attention region. Then out-proj + LN1, bf16
FFN (w1 and w2 streamed), LN2, output DMA.
"""
import numpy as np
import ml_dtypes
import concourse.bass as bass
from concourse import bacc
import concourse.mybir as mybir
import concourse.tile as tile
from concourse.bass import ts
from concourse.bass_utils import run_bass_kernel_spmd

P = 128
B, S, D, H, W, F = 2, 2048, 1024, 16, 64, 4096
DC = D // P            # 8 128-feature chunks
FC = F // P            # 32
FCH = D // W           # 16 64-feature chunks (one per head)
TC = S // P            # 16 key-token chunks
SCH = 512              # tokens per core
EPS = 1e-12
SCALE = 1.0 / np.sqrt(np.float32(W))
DEFER = 6              # attn-V lags scores by this many heads
NPROBS = DEFER + 1

F32 = mybir.dt.float32
F32R = mybir.dt.float32r
FP8 = mybir.dt.float8e4
BF16 = mybir.dt.bfloat16
DR = mybir.MatmulPerfMode.DoubleRow

_cache = {}


def _layer_norm(nc, ppb, ps_u, ps_v, pool, invd, ru2, gnb, src, sq, dst, tag,
                out_dma=None):
    """src/sq: [P, DC, SCH] f32r (feature on partitions); LN over features.
    ps_u/ps_v: [1, SCH] PSUM stats already accumulated by the producer loop.
    dst = src*A - B with rank-1 A = g (x) rstd, B = g (x) u*rstd - b (x) 1.
    gnb is a [2, D] tile with rows (g, -b); ru2 is a persistent [2, SCH]
    tile whose row 1 is ones (row 0 gets u*rstd here). out_dma(dc) is
    called right after dst[:, dc] is written."""
    at = mybir.ActivationFunctionType
    u = pool.tile([1, SCH], F32R, tag=tag + "u", name="u")
    var = pool.tile([1, SCH], F32, tag=tag + "var", name="var")
    sd = pool.tile([1, SCH], F32, tag=tag + "sd", name="sd")
    rstd = pool.tile([1, SCH], F32R, tag=tag + "rstd", name="rstd")
    nc.vector.tensor_copy(u[:], ps_u[:])
    nc.vector.tensor_tensor(var[:], u[:], u[:], mybir.AluOpType.mult)
    nc.vector.tensor_tensor(var[:], ps_v[:], var[:], mybir.AluOpType.subtract)
    nc.scalar.activation(sd[:], var[:], at.Sqrt, bias=EPS)
    nc.vector.reciprocal(rstd[:], sd[:])
    nc.vector.tensor_tensor(ru2[0:1], u[:], rstd[:], mybir.AluOpType.mult)
    for dc in range(DC):
        ps_a = ppb.tile([P, SCH], F32, tag="ps", name="ps_a")
        ps_b = ppb.tile([P, SCH], F32, tag="ps", name="ps_b")
        nc.tensor.matmul(ps_a[:], gnb[0:1, ts(dc, P)], rstd[:],
                         start=True, stop=True)
        nc.tensor.matmul(ps_b[:], gnb[:, ts(dc, P)], ru2[:],
                         start=True, stop=True)
        t = pool.tile([P, SCH], F32, tag="lnt", bufs=2, name="lnt")
        nc.vector.tensor_tensor(t[:], src[:, dc], ps_a[:], mybir.AluOpType.mult)
        nc.vector.tensor_tensor(dst[:, dc], t[:], ps_b[:],
                                mybir.AluOpType.subtract)
        if out_dma is not None:
            out_dma(dc)


def _build(masked=False):
    at = mybir.ActivationFunctionType
    nc = bacc.Bacc("TRN2", target_bir_lowering=False)

    xT_d = nc.dram_tensor("xT8", [P, DC, S], FP8, kind="ExternalInput")
    xq_d = nc.dram_tensor("xq8", [P, DC, SCH], FP8, kind="ExternalInput")
    xs_d = nc.dram_tensor("xs", [P, DC, SCH], F32R, kind="ExternalInput")
    wq_d = nc.dram_tensor("wq8", [P, DC, D], FP8, kind="ExternalInput")
    wk_d = nc.dram_tensor("wk8", [P, DC, D], FP8, kind="ExternalInput")
    wv_d = nc.dram_tensor("wv8", [P, DC, D], FP8, kind="ExternalInput")
    wo_d = nc.dram_tensor("wo8", [P, DC, D], FP8, kind="ExternalInput")
    w1_d = nc.dram_tensor("w1b", [P, DC, F], BF16, kind="ExternalInput")
    w2_d = nc.dram_tensor("w2b", [P, FC, D], BF16, kind="ExternalInput")
    bq_d = nc.dram_tensor("bq64", [W, FCH], F32, kind="ExternalInput")
    bk_d = nc.dram_tensor("bk64", [W, FCH], F32, kind="ExternalInput")
    bv_d = nc.dram_tensor("bvr", [1, D], F32R, kind="ExternalInput")
    bo_d = nc.dram_tensor("bo", [P, DC], F32, kind="ExternalInput")
    bf1_d = nc.dram_tensor("bf1", [P, FC], F32, kind="ExternalInput")
    bf2_d = nc.dram_tensor("bf2", [P, DC], F32, kind="ExternalInput")
    gnb1_d = nc.dram_tensor("gnb1", [2, D], F32R, kind="ExternalInput")
    gnb2_d = nc.dram_tensor("gnb2", [2, D], F32R, kind="ExternalInput")
    invd_d = nc.dram_tensor("invd", [P, 1], F32R, kind="ExternalInput")
    ones5_d = nc.dram_tensor("ones512", [1, SCH], F32R, kind="ExternalInput")
    mb_d = nc.dram_tensor("mb", [P, TC], F32, kind="ExternalInput") if masked else None
    out_d = nc.dram_tensor("outT", [P, DC, SCH], F32, kind="ExternalOutput")

    # V tiles (tcl, grp) packed into the first head-iterations
    v_tiles = [(tcl, g) for tcl in range(TC) for g in range(2)]
    v_sched = {0: v_tiles[0:6], 1: v_tiles[6:12], 2: v_tiles[12:17],
               3: v_tiles[17:22], 4: v_tiles[22:27], 5: v_tiles[27:32]}

    with nc.allow_low_precision(reason="fp8/bf16 by design"), \
         tile.TileContext(nc) as tc:
        with tc.tile_pool(name="small", bufs=1) as small, \
             tc.tile_pool(name="pps", bufs=2, space="PSUM") as pps, \
             tc.tile_pool(name="ppk", bufs=2, space="PSUM") as ppk:
            # ---- long-lived tiles, reverse order of death ----
            hT, hT_free = tc.tile([P, DC, SCH], FP8, name="hT")
            wo8, wo8_free = tc.tile([P, DC, D], FP8, name="wo8")
            xs2, xs2_free = tc.tile([P, DC, SCH], F32R, name="xs2")
            kT, kT_free = tc.tile([W, FCH, S], FP8, name="kT")
            qT, qT_free = tc.tile([W, FCH, SCH], FP8, name="qT")
            vA, vA_free = tc.tile([P, TC, D], FP8, name="vA")

            ones8 = small.tile([P, 2, W], FP8)
            onesr = small.tile([1, P], F32R)
            bvb = small.tile([P, D], F32)
            bk_sb = small.tile([W, FCH], F32)
            bq_sb = small.tile([W, FCH], F32)
            bo_sb = small.tile([P, DC], F32)
            bf1_sb = small.tile([P, FC], F32)
            bf2_sb = small.tile([P, DC], F32)
            invd = small.tile([P, 1], F32R)
            gnb1 = small.tile([2, D], F32R)
            ru2 = small.tile([2, SCH], F32R)
            epsc = small.tile([P, 1], F32)
            mb_sb = small.tile([P, TC], F32) if masked else None

            nc.vector.memset(epsc[:], EPS)
            nc.const_aps.aps[(F32, EPS)] = epsc[:]
            nc.vector.memset(ones8[:], 1.0)
            nc.sync.dma_start(onesr[:], ones5_d[:, 0:P])
            nc.sync.dma_start(ru2[1:2], ones5_d[:])

            with tc.tile_pool(name="wpool", bufs=1) as wpool:
                wk8 = wpool.tile([P, DC, D], FP8)
                wv8 = wpool.tile([P, DC, D], FP8)
                xT8 = wpool.tile([P, DC, S], FP8)

                # ---- Q projection upfront + bvb = ones (x) bv ----
                with tc.tile_pool(name="wqpool", bufs=1) as wqpool:
                    wq8 = wqpool.tile([P, DC, D], FP8)
                    xq8 = wqpool.tile([P, DC, SCH], FP8)
                    bv_row = wqpool.tile([1, D], F32R)
                    # Q-path DMAs first on their queues
                    nc.sync.dma_start(wq8[:, :, 0:512], wq_d[:, :, 0:512])
                    nc.gpsimd.dma_start(xq8[:], xq_d[:])
                    nc.scalar.dma_start(bq_sb[:], bq_d[:])
                    nc.scalar.dma_start(bv_row[:], bv_d[:])
                    nc.sync.dma_start(wq8[:, :, 512:], wq_d[:, :, 512:])
                    nc.scalar.dma_start(wk8[:], wk_d[:])
                    nc.gpsimd.dma_start(xT8[:, :, 0:1024], xT_d[:, :, 0:1024])
                    nc.gpsimd.dma_start(xT8[:, :, 1024:], xT_d[:, :, 1024:])
                    nc.scalar.dma_start(bk_sb[:], bk_d[:])
                    nc.sync.dma_start(wv8[:], wv_d[:])
                    nc.sync.dma_start(wo8[:], wo_d[:])
                    nc.sync.dma_start(bo_sb[:], bo_d[:])
                    nc.sync.dma_start(bf1_sb[:], bf1_d[:])
                    nc.sync.dma_start(bf2_sb[:], bf2_d[:])
                    nc.sync.dma_start(invd[:], invd_d[:])
                    nc.sync.dma_start(gnb1[:], gnb1_d[:])
                    if masked:
                        nc.sync.dma_start(mb_sb[:], mb_d[:])
                    for dcw in range(DC):
                        nc.gpsimd.dma_start(xs2[:, dcw], xs_d[:, dcw])
                    # warm the Exp table while DMAs land
                    wrm = wqpool.tile([1, 1], F32)
                    nc.scalar.activation(wrm[:], epsc[0:1, :], at.Exp)
                    for hq in range(FCH):
                        psq = ppk.tile([W, 1024], F32, tag="pk", name="psq")[:, 0:SCH]
                        for g in range(4):
                            nc.tensor.matmul(psq[:],
                                             wq8[:, 2 * g:2 * g + 2, ts(hq, W)],
                                             xq8[:, 2 * g:2 * g + 2, :],
                                             start=(g == 0), stop=(g == 3),
                                             perf_mode=DR)
                        nc.vector.tensor_scalar(qT[:, hq], psq[:],
                                                bq_sb[:, hq:hq + 1], None,
                                                mybir.AluOpType.add)
                    psbv = pps.tile([P, D], F32, tag="ps", name="psbv")
                    nc.tensor.matmul(psbv[:, 0:512], onesr[:],
                                     bv_row[:, 0:512], start=True, stop=True)
                    nc.tensor.matmul(psbv[:, 512:], onesr[:],
                                     bv_row[:, 512:], start=True, stop=True)
                    nc.vector.tensor_copy(bvb[:], psbv[:])

                # ===== fused K/Q/scores/exp/V + deferred attn-V =====
                probs_tiles = {}
                with tc.tile_pool(name="prp", bufs=NPROBS) as prp, \
                     tc.tile_pool(name="stage", bufs=2) as stp:

                    def attn_units(hh):
                        """attn-V + Z + normalize for head hh (vA complete),
                        as schedulable units. pso/psz share one 2-bank slot,
                        allocated lazily at first use so the ring slot is not
                        claimed before other pool users emitted in between."""
                        pr = probs_tiles.pop(hh)
                        st_ = {}

                        def get_psoz():
                            if "t" not in st_:
                                st_["t"] = ppk.tile([W, D], F32, tag="pk",
                                                    name="psoz")
                            return st_["t"]

                        def mk_av(k0, k1, st, sp):
                            def u():
                                pso = get_psoz()[:, 0:SCH]
                                for kcp in range(k0, k1):
                                    nc.tensor.matmul(
                                        pso, vA[:, 2 * kcp:2 * kcp + 2,
                                                ts(hh, W)],
                                        pr[:, 2 * kcp:2 * kcp + 2, :],
                                        start=(st and kcp == k0),
                                        stop=(sp and kcp == k1 - 1),
                                        perf_mode=DR)
                            return u

                        def mk_z():
                            def u():
                                psz = get_psoz()[:, SCH:]
                                for kcp in range(TC // 2):
                                    nc.tensor.matmul(
                                        psz, ones8[:],
                                        pr[:, 2 * kcp:2 * kcp + 2, :],
                                        start=(kcp == 0),
                                        stop=(kcp == TC // 2 - 1),
                                        perf_mode=DR)
                            return u

                        def norm():
                            psoz = get_psoz()
                            pso, psz = psoz[:, 0:SCH], psoz[:, SCH:]
                            rzb = stp.tile([W, SCH], BF16, tag="rz", name="rzb")
                            nc.vector.reciprocal(rzb[:], psz)
                            hc, hp = hh // 2, W * (hh % 2)
                            if hp == 0:
                                nc.vector.tensor_tensor(hT[0:W, hc], pso,
                                                        rzb[:],
                                                        mybir.AluOpType.mult)
                            else:
                                hst = stp.tile([W, SCH], FP8, tag="hst",
                                               bufs=1, name="hst")
                                nc.vector.tensor_tensor(hst[:], pso, rzb[:],
                                                        mybir.AluOpType.mult)
                                nc.gpsimd.dma_start(hT[hp:hp + W, hc], hst[:])
                        return [mk_av(0, 4, True, False),
                                mk_av(4, 8, False, True), mk_z(), norm]

                    def k_half(hh, th):
                        # kT[:, hh, 1024*th:+1024]: g-outer so the wk8
                        # stationary is reused across both sub-regions
                        psk = ppk.tile([W, 1024], F32, tag="pk", name="psk")
                        for g in range(4):
                            for sub in range(2):
                                tok = 1024 * th + 512 * sub
                                nc.tensor.matmul(
                                    psk[:, ts(sub, 512)],
                                    wk8[:, 2 * g:2 * g + 2, ts(hh, W)],
                                    xT8[:, 2 * g:2 * g + 2, tok:tok + 512],
                                    start=(g == 0), stop=(g == 3),
                                    perf_mode=DR)
                        nc.vector.tensor_scalar(
                            kT[:, hh, ts(th, 1024)], psk[:],
                            bk_sb[:, hh:hh + 1], None, mybir.AluOpType.add)

                    def v_tile(tcl, grp):
                        psv = ppk.tile([W, D], F32, tag="pk", name="psv")
                        t0 = tcl * P + grp * W
                        for g in range(4):
                            for half in range(2):
                                nc.tensor.matmul(
                                    psv[:, ts(half, 512)],
                                    xT8[:, 2 * g:2 * g + 2, t0:t0 + W],
                                    wv8[:, 2 * g:2 * g + 2, ts(half, 512)],
                                    start=(g == 0), stop=(g == 3),
                                    perf_mode=DR)
                        if grp == 0:
                            nc.vector.tensor_tensor(vA[0:W, tcl, :], psv[:],
                                                    bvb[0:W, :],
                                                    mybir.AluOpType.add)
                        else:
                            vst = stp.tile([W, D], FP8, tag="vst", name="vst")
                            nc.vector.tensor_tensor(vst[:], psv[:],
                                                    bvb[0:W, :],
                                                    mybir.AluOpType.add)
                            nc.gpsimd.dma_start(vA[W:P, tcl, :], vst[:])

                    k_half(0, 0)
                    k_half(0, 1)
                    for h in range(H):
                        # filler units to slot between scores/exp pairs
                        fillers = []
                        if h + 1 < H:
                            fillers.append(lambda th=0: k_half(h + 1, 0))
                            fillers.append(lambda th=1: k_half(h + 1, 1))
                        for (tcl, grp) in v_sched.get(h, []):
                            fillers.append(
                                lambda a=tcl, b=grp: v_tile(a, b))
                        if h >= DEFER:
                            fillers.extend(attn_units(h - DEFER))
                        # scores + exp for head h, fillers interleaved
                        pr = prp.tile([P, TC, SCH], FP8, tag="probs", name="pr")
                        probs_tiles[h] = pr
                        for kcp in range(TC // 2):
                            pss = pps.tile([P, 2, 512], F32, tag="ps",
                                           name="pss")
                            for j in range(2):
                                kc = 2 * kcp + j
                                nc.tensor.matmul(pss[:, j], kT[:, h, ts(kc, P)],
                                                 qT[:, h], start=True, stop=True)
                            if masked:
                                for j in range(2):
                                    kc = 2 * kcp + j
                                    nc.scalar.activation(
                                        pr[:, kc, :], pss[:, j], at.Exp,
                                        bias=mb_sb[:, kc:kc + 1],
                                        scale=float(SCALE))
                            else:
                                nc.scalar.activation(
                                    pr[:, 2 * kcp:2 * kcp + 2, :], pss[:],
                                    at.Exp, scale=float(SCALE))
                            while fillers and len(fillers) >= TC // 2 - kcp:
                                fillers.pop(0)()
                        for f in fillers:
                            f()
                    for hh in range(H - DEFER, H):
                        for f in attn_units(hh):
                            f()
            vA_free()
            qT_free()
            kT_free()

            # ================= out-proj + residual + LN1 =================
            h1T, h1T_free = tc.tile([P, DC, SCH], BF16, name="h1T")
            r1T, r1T_free = tc.tile([P, DC, SCH], F32R, name="r1T")
            sq1, sq1_free = tc.tile([P, DC, SCH], F32R, name="sq1")
            # warm Act tables needed by the next phases
            wrm2 = small.tile([1, 4], F32)
            nc.scalar.activation(wrm2[:, 0:1], epsc[0:1, :], at.Square)
            nc.scalar.activation(wrm2[:, 1:2], epsc[0:1, :], at.Sqrt)
            nc.scalar.activation(wrm2[:, 2:3], epsc[0:1, :], at.Gelu)
            ps_u1 = ppk.tile([1, SCH], F32, tag="pk", name="ps_u1")
            ps_v1 = ppk.tile([1, SCH], F32, tag="pk", name="ps_v1")
            for dp in range(DC):
                psr = pps.tile([P, SCH], F32, tag="ps", name="psr")
                for dc in range(DC):
                    nc.tensor.matmul(psr[:], wo8[:, dc, ts(dp, P)],
                                     hT[:, dc],
                                     start=(dc == 0), stop=(dc == DC - 1))
                nc.vector.tensor_scalar(r1T[:, dp], psr[:],
                                        bo_sb[:, dp:dp + 1], None,
                                        mybir.AluOpType.add)
                nc.vector.tensor_tensor(r1T[:, dp], r1T[:, dp], xs2[:, dp],
                                        mybir.AluOpType.add)
                nc.scalar.activation(sq1[:, dp], r1T[:, dp], at.Square)
                nc.tensor.matmul(ps_u1[:], invd[:], r1T[:, dp],
                                 start=(dp == 0), stop=(dp == DC - 1))
                nc.tensor.matmul(ps_v1[:], invd[:], sq1[:, dp],
                                 start=(dp == 0), stop=(dp == DC - 1))
            _layer_norm(nc, pps, ps_u1, ps_v1, small, invd, ru2, gnb1,
                        r1T, sq1, h1T, "ln1")
            sq1_free()
            r1T_free()
            nc.sync.dma_start(gnb1[:], gnb2_d[:])

            # ================= FFN =================
            g1T, g1T_free = tc.tile([P, FC, SCH], BF16, name="g1T")
            with tc.tile_pool(name="pf1", bufs=2) as pf1:
                for fcp in range(FC // 2):
                    w1t = pf1.tile([P, DC, 2 * P], BF16, tag="w1t", name="w1t")
                    nc.sync.dma_start(w1t[:], w1_d[:, :, ts(fcp, 2 * P)])
                    for j in range(2):
                        fc = 2 * fcp + j
                        psg = pps.tile([P, SCH], F32, tag="ps", name="psg")
                        for dc in range(DC):
                            nc.tensor.matmul(psg[:], w1t[:, dc, ts(j, P)],
                                             h1T[:, dc],
                                             start=(dc == 0),
                                             stop=(dc == DC - 1))
                        nc.scalar.activation(g1T[:, fc], psg[:], at.Gelu,
                                             bias=bf1_sb[:, fc:fc + 1])
            r2T, r2T_free = tc.tile([P, DC, SCH], F32R, name="r2T")
            sq2, sq2_free = tc.tile([P, DC, SCH], F32R, name="sq2")
            ps_u2 = ppk.tile([1, SCH], F32, tag="pk", name="ps_u2")
            ps_v2 = ppk.tile([1, SCH], F32, tag="pk", name="ps_v2")
            with tc.tile_pool(name="pw2", bufs=2) as pw2:
                for dp in range(DC):
                    w2t = pw2.tile([P, FC, P], BF16, tag="w2t", name="w2t")
                    nc.sync.dma_start(w2t[:, 0:FC // 2],
                                      w2_d[:, 0:FC // 2, ts(dp, P)])
                    nc.scalar.dma_start(w2t[:, FC // 2:],
                                        w2_d[:, FC // 2:, ts(dp, P)])
                    psf = pps.tile([P, SCH], F32, tag="ps", name="psf")
                    for fc in range(FC):
                        nc.tensor.matmul(psf[:], w2t[:, fc], g1T[:, fc],
                                         start=(fc == 0), stop=(fc == FC - 1))
                    nc.vector.tensor_scalar(r2T[:, dp], psf[:],
                                            bf2_sb[:, dp:dp + 1], None,
                                            mybir.AluOpType.add)
                    nc.vector.tensor_tensor(r2T[:, dp], r2T[:, dp],
                                            h1T[:, dp], mybir.AluOpType.add)
                    nc.scalar.activation(sq2[:, dp], r2T[:, dp], at.Square)
                    nc.tensor.matmul(ps_u2[:], invd[:], r2T[:, dp],
                                     start=(dp == 0), stop=(dp == DC - 1))
                    nc.tensor.matmul(ps_v2[:], invd[:], sq2[:, dp],
                                     start=(dp == 0), stop=(dp == DC - 1))

            # ================= LN2 + out =================
            oT, oT_free = tc.tile([P, DC, SCH], F32, name="oT")

            def _odma(dc):
                eng = nc.sync if dc % 2 == 0 else nc.scalar
                eng.dma_start(out_d[:, dc], oT[:, dc])
            _layer_norm(nc, pps, ps_u2, ps_v2, small, invd, ru2, gnb1,
                        r2T, sq2, oT, "ln2", out_dma=_odma)
            oT_free()
            sq2_free()
            r2T_free()
            g1T_free()
            h1T_free()
            xs2_free()
            wo8_free()
            hT_free()

    nc.compile()
    return nc


def kernel(**inputs):
    x = np.asarray(inputs["x"], dtype=np.float32)
    mask = np.asarray(inputs["mask"])
    f = {k: np.asarray(inputs[k], dtype=np.float32) for k in
         ["wq", "bq", "wk", "bk", "wv", "bv", "wo", "bo", "g1", "b1",
          "w1", "bf1", "w2", "bf2", "g2", "b2"]}

    masked = not bool(np.all(mask == 1))
    key = ("nc", masked)
    if key not in _cache:
        _cache[key] = _build(masked)
    nc = _cache[key]
    _cache["nc"] = nc  # test.py reads this for TimelineSim

    def wlay(w, pc):  # [K, M] -> [P, K//P, M]
        return np.ascontiguousarray(w.reshape(pc, P, w.shape[1]).transpose(1, 0, 2))

    def blay(b):      # [M] -> [P, M//P]
        return np.ascontiguousarray(b.reshape(-1, P).T)

    fp8 = ml_dtypes.float8_e4m3fn
    bf16 = ml_dtypes.bfloat16
    shared = {
        "wq8": wlay(f["wq"], DC).astype(fp8),
        "wk8": wlay(f["wk"], DC).astype(fp8),
        "wv8": wlay(f["wv"], DC).astype(fp8),
        "wo8": wlay(f["wo"], DC).astype(fp8),
        "w1b": wlay(f["w1"], DC).astype(bf16),
        "w2b": wlay(f["w2"], FC).astype(bf16),
        "invd": np.full((P, 1), 1.0 / D, np.float32),
        "ones512": np.ones((1, SCH), np.float32),
        "gnb1": np.ascontiguousarray(
            np.stack([f["g1"], -f["b1"]]).astype(np.float32)),
        "gnb2": np.ascontiguousarray(
            np.stack([f["g2"], -f["b2"]]).astype(np.float32)),
        "bq64": np.ascontiguousarray(f["bq"].reshape(FCH, W).T),
        "bk64": np.ascontiguousarray(f["bk"].reshape(FCH, W).T),
        "bvr": f["bv"].reshape(1, D),
        "bo": blay(f["bo"]), "bf1": blay(f["bf1"]), "bf2": blay(f["bf2"]),
    }

    in_maps = []
    for c in range(8):
        b, sq = c // 4, c % 4
        xTb = np.ascontiguousarray(x[b].T.reshape(DC, P, S).transpose(1, 0, 2))
        m = dict(shared)
        m["xT8"] = xTb.astype(fp8)
        xsl = np.ascontiguousarray(xTb[:, :, sq * SCH:(sq + 1) * SCH])
        m["xq8"] = xsl.astype(fp8)
        m["xs"] = xsl
        if masked:
            mbias = (-10000.0 * (1.0 - mask[b].astype(np.float32)))
            m["mb"] = np.ascontiguousarray(mbias.reshape(TC, P).T)
        in_maps.append(m)

    res = run_bass_kernel_spmd(nc, in_maps, core_ids=list(range(8)))
    _cache["last_res"] = res

    out = np.empty((B, S, D), np.float32)
    for c in range(8):
        b, sq = c // 4, c % 4
        oT = res.results[c]["outT"]  # [P, DC, SCH]
        out[b, sq * SCH:(sq + 1) * SCH, :] = oT.transpose(2, 1, 0).reshape(SCH, D)
    return out
